# revision 22
# baseline (speedup 1.0000x reference)
"""NT-Xent / SimCLR contrastive loss on 8 Trainium2 NeuronCores.

v6: fp8 DoubleRow matmuls, u16-packed fp8 xbar transpose, (p t) row
layout for single-descriptor DMAs, hoisted loads, DVE/ACT split scaling.

Strategy (data-parallel over rows of the concatenated representations):
  - Host: reps = concat(z_i, z_j) -> [8192, 512] fp32. Core i receives
    reps rolled by -1024*i rows so that *its* 1024 rows sit at rows
    0..1023. SPMD-identical program; positives at col = row + 4096.
  - Row layout "(p t)": group row p*8 + t lives at partition p, tile t,
    so each partition's 8 rows are CONTIGUOUS 16KB in DRAM -> one
    descriptor per partition per group load (8x fewer than "(t p)"),
    and the scratch bounce store is one 2KB run per partition. All
    loads are issued up front (rows pool holds all 8 groups).
  - Per group: n2 via DVE stt square+rowsum; inv = S/sqrt(n2) via ACT
    Ln+Exp (S=16 scales rows into fp8 range); normed fp8 = rows * inv,
    alternating DVE tensor_scalar / ACT Copy per tile; u16-packed
    feature pairs bounce through DRAM scratch (natural row order) and
    xbar-transpose into repsT[kp][g] [128, 1024] u16 (col q = group
    row q; partition c = features 256*kp + {2c, 2c+1} interleaved).
  - Phase B: DoubleRow fp8 matmuls ([128, 2, N] APs, dim1 = the byte
    interleave; lhsT = planar deinterleaved own columns). exp(2*sim) =
    ACT Exp(scale=2/S^2) with fused row-sum. Positives diagonal taken
    from PSUM (natural col order) with an identity-mask stt.
  - Self-similarity: denom = rowsum - e^2 (constant; sim_ii == 1 up to
    fp8 quantization, error ~4e-6 relative on the loss).
  - A dependency-chained dummy LDWEIGHTS per group keeps the PE HAM
    clock gate open through phase A.
  - Host: loss = sum(core partials) / 8192.
"""

import sys
import threading
from unittest import mock

sys.path.insert(0, "/opt/trn_rl_repo")

import numpy as np  # noqa: E402

import concourse.tile as tile  # noqa: E402
from concourse import bacc, mybir  # noqa: E402
from concourse.bass_utils import run_bass_kernel_spmd  # noqa: E402
from concourse.hw_specs import get_activation_tables  # noqa: E402
from concourse.masks import make_identity  # noqa: E402
from contextlib import ExitStack  # noqa: E402

P = 128
D = 512
TWO_N = 8192
N_CORES = 8
ROWS_PER_CORE = TWO_N // N_CORES  # 1024
T_INV = 2.0  # 1 / temperature (0.5)
S = 16.0  # fp8 range scale: normed rows stored as S * x / ||x||
LOGIT_SCALE = T_INV / (S * S)  # psum -> exp argument
E_SELF = float(np.exp(T_INV))  # exp(T_INV * sim_ii), sim_ii == 1

NG = 8  # row/column groups
CB = TWO_N // NG  # 1024 rows per group
TPG = CB // P  # 8 [128, 512] row tiles per group
MB = ROWS_PER_CORE // P  # 8 m-blocks of 128 rows
NKP = 2  # feature-pair chunks (256 features each)
NPB = 4  # psum blocks of 2048 columns (2 groups each)

FP32 = mybir.dt.float32
BF16 = mybir.dt.bfloat16
FP8 = mybir.dt.float8e4
U16 = mybir.dt.uint16
AF = mybir.ActivationFunctionType
ALU = mybir.AluOpType
AX = mybir.AxisListType
DR = mybir.MatmulPerfMode.DoubleRow


def _filtered_activation_tables(arch):
    """Steer every Exp/Ln/Copy activation to the one table set containing
    both Exp and Ln, so the table-load pass cannot thrash between sets."""
    tables = get_activation_tables(arch)
    target = None
    for name, funcs in tables.items():
        if AF.Exp in funcs and AF.Ln in funcs:
            target = name
            break
    if target is None:
        return tables
    steer = {AF.Exp, AF.Ln, AF.Copy, AF.Identity}
    return {
        name: (funcs if name == target else funcs - steer)
        for name, funcs in tables.items()
    }


def _build_kernel():
    nc = bacc.Bacc("TRN2", target_bir_lowering=False, debug=False,
                   num_devices=N_CORES)
    reps = nc.dram_tensor("reps", [TWO_N, D], FP32, kind="ExternalInput").ap()
    out = nc.dram_tensor("out", [1, 1], FP32, kind="ExternalOutput").ap()

    with tile.TileContext(nc) as tc, ExitStack() as ctx:
        rows_pool = ctx.enter_context(tc.tile_pool(name="rows", bufs=1))
        normed_pool = ctx.enter_context(tc.tile_pool(name="normed", bufs=2))
        sq_pool = ctx.enter_context(tc.tile_pool(name="sq", bufs=2))
        stats_pool = ctx.enter_context(tc.tile_pool(name="stats", bufs=1))
        repsT_pool = ctx.enter_context(tc.tile_pool(name="repsT", bufs=1))
        dram_pool = ctx.enter_context(
            tc.tile_pool(name="scratch", bufs=NKP * NG, space="DRAM"))
        psum_pool = ctx.enter_context(
            tc.tile_pool(name="psum", bufs=2, space="PSUM"))
        exp_pool = ctx.enter_context(tc.tile_pool(name="exp", bufs=2))
        junk_pool = ctx.enter_context(tc.tile_pool(name="junk", bufs=2))
        epi_pool = ctx.enter_context(tc.tile_pool(name="epi", bufs=1))

        # --- constants -----------------------------------------------------
        ident = stats_pool.tile([P, P], FP32, tag="ident", name="ident")
        make_identity(nc, ident[:])
        ones = stats_pool.tile([P, 1], FP32, tag="ones", name="ones")
        nc.gpsimd.memset(ones[:], 1.0)

        # accumulators for the main loop
        rs_all = stats_pool.tile([P, MB * NPB], FP32, tag="rs", name="rs_all")
        pos = stats_pool.tile([P, MB], FP32, tag="pos", name="pos")

        # repsT[kp][g]: [128, 1024] u16 — feature-pair chunk kp (features
        # 256*kp + {2c, 2c+1} byte-interleaved at partition c), group col
        # q = group row q (natural order).
        repsT = [[repsT_pool.tile([P, CB], U16, tag=f"rT{kp}_{g}",
                                  name=f"repsT_{kp}_{g}")
                  for g in range(NG)]
                 for kp in range(NKP)]
        # repsT0[kp]: [128, 2048] fp8, planar deinterleave of own columns:
        # layout "p (two q)" so the DoubleRow lhsT slice [128, 2, 128] has
        # a contiguous fast dim.
        repsT0 = [repsT_pool.tile([P, 2 * ROWS_PER_CORE], FP8, tag=f"rTz_{kp}",
                                  name=f"repsT0_{kp}")
                  for kp in range(NKP)]

        # --- all group loads up front (no head-of-line blocking) -----------
        rows_tiles = []
        for g in range(NG):
            rows_g = rows_pool.tile([P, TPG * D], FP32, tag=f"rows{g}",
                                    name=f"rows_{g}")
            rows_tiles.append(rows_g)
            src = reps[g * CB:(g + 1) * CB, :].rearrange(
                "(p t) d -> p t d", t=TPG)
            if g == 0:
                # halves: square pass starts after 8KB/partition arrives
                for h in range(2):
                    hs = TPG // 2
                    nc.sync.dma_start(
                        out=rows_g[:, h * hs * D:(h + 1) * hs * D].rearrange(
                            "p (t d) -> p t d", d=D),
                        in_=src[:, h * hs:(h + 1) * hs, :])
            else:
                nc.sync.dma_start(
                    out=rows_g[:].rearrange("p (t d) -> p t d", d=D), in_=src)

        # --- phase A: normalize rows to fp8, transpose via u16 xbar --------
        for g in range(NG):
            rows_g = rows_tiles[g]
            n2 = stats_pool.tile([P, TPG], FP32, tag="n2", bufs=2,
                                 name=f"n2_{g}")
            for t in range(TPG):
                sq = sq_pool.tile([P, D], BF16, tag="sq", name=f"sq_{g}_{t}")
                rt = rows_g[:, t * D:(t + 1) * D]
                nc.vector.scalar_tensor_tensor(
                    out=sq[:], in0=rt, scalar=1.0, in1=rt,
                    op0=ALU.mult, op1=ALU.mult, accum_out=n2[:, t:t + 1])
            # inv = S * n2**-0.5 = exp(-0.5 * ln(n2 / S^2))
            lnn = stats_pool.tile([P, TPG], FP32, tag="lnn", bufs=2,
                                  name=f"lnn_{g}")
            nc.scalar.activation(lnn[:], n2[:], AF.Ln, scale=1.0 / (S * S))
            inv = stats_pool.tile([P, TPG], FP32, tag="inv", bufs=2,
                                  name=f"inv_{g}")
            nc.scalar.activation(inv[:], lnn[:], AF.Exp, scale=-0.5)

            normed_g = normed_pool.tile([P, TPG * D], FP8, tag="normed",
                                        name=f"normed_{g}")
            for t in range(TPG):
                # split row-scaling between DVE and GpSimd (both idle-ish
                # in phase A; keeps the per-group feed pace off one engine)
                if t % 2 == 0:
                    nc.vector.tensor_scalar(
                        out=normed_g[:, t * D:(t + 1) * D],
                        in0=rows_g[:, t * D:(t + 1) * D],
                        scalar1=inv[:, t:t + 1], scalar2=None, op0=ALU.mult)
                else:
                    nc.gpsimd.tensor_scalar_mul(
                        normed_g[:, t * D:(t + 1) * D],
                        rows_g[:, t * D:(t + 1) * D],
                        inv[:, t:t + 1])
            # HAM keep-warm: dependency-chained LDWEIGHTS (no PSUM write)
            nc.tensor.ldweights(weights=normed_g[:, 0:P], perf_mode=None)

            # u16 view: adjacent feature pairs -> 2-byte units for the xbar.
            # Natural order: scr row p*8 + t = normed (p, t) = group row.
            # ONE contiguous store per group (4KB/partition run on both
            # sides); the two transposes read kp column-slices of it.
            nview = normed_g[:].bitcast(U16).rearrange(
                "p (t e) -> p t e", e=D // 2)
            scr = dram_pool.tile([CB, NKP * P], U16, tag=f"scr_{g}",
                                 name=f"scr_{g}")
            nc.sync.dma_start(
                out=scr[:].rearrange("(p t) c -> p t c", p=P), in_=nview)
            for kp in range(NKP):
                nc.sync.dma_start_transpose(
                    repsT[kp][g][:], scr[:, kp * P:(kp + 1) * P])
            if g == 0:
                # deinterleave own columns into planar lhsT layout:
                # dst[c, i*1024 + q] = src byte 2q + i
                for kp in range(NKP):
                    nc.vector.tensor_copy(
                        repsT0[kp][:].rearrange(
                            "p (two q) -> p two q", two=2),
                        repsT[kp][0][:].bitcast(FP8).rearrange(
                            "p (q two) -> p two q", two=2))

        # --- phase B: DoubleRow similarity matmuls + softmax stats ---------
        for pb in range(NPB):
            for m in range(MB):
                ps = psum_pool.tile([P, 2 * CB], FP32, tag="ps",
                                    name=f"ps_{pb}_{m}")
                for kp in range(NKP):
                    lhsT = repsT0[kp][:].rearrange(
                        "p (two mj) -> p two mj", two=2)[
                        :, :, m * P:(m + 1) * P]
                    for half in range(2):
                        rhs_g = repsT[kp][2 * pb + half][:].bitcast(
                            FP8).rearrange("p (n two) -> p two n", two=2)
                        for ns in range(2):
                            nc.tensor.matmul(
                                ps[:, half * CB + ns * 512:
                                   half * CB + (ns + 1) * 512],
                                lhsT=lhsT,
                                rhs=rhs_g[:, :, ns * 512:(ns + 1) * 512],
                                start=(kp == 0), stop=(kp == NKP - 1),
                                perf_mode=DR, skip_group_check=True)
                et = exp_pool.tile([P, 2 * CB], BF16, tag="et",
                                   name=f"et_{pb}_{m}")
                nc.scalar.activation(
                    et[:], ps[:], AF.Exp, scale=LOGIT_SCALE,
                    accum_out=rs_all[:, m * NPB + pb:m * NPB + pb + 1])
                if pb == 2:
                    # positives: global col = 4096 + row -> group 4 (first
                    # half of this psum block), natural in-group col = row.
                    junk = junk_pool.tile([P, P], FP32, tag="junk",
                                          name=f"junk_p_{m}")
                    nc.vector.scalar_tensor_tensor(
                        out=junk[:], in0=ps[:, m * P:(m + 1) * P],
                        scalar=1.0, in1=ident[:],
                        op0=ALU.mult, op1=ALU.mult,
                        accum_out=pos[:, m:m + 1])

        # --- epilogue ------------------------------------------------------
        sums = epi_pool.tile([P, MB], FP32, tag="sums", name="sums")
        nc.vector.tensor_reduce(
            sums[:], rs_all[:].rearrange("p (m b) -> p m b", b=NPB),
            axis=AX.X, op=ALU.add)
        denom = epi_pool.tile([P, MB], FP32, tag="denom", name="denom")
        nc.vector.tensor_scalar_add(denom[:], sums[:], -E_SELF)
        ld = epi_pool.tile([P, MB], FP32, tag="ld", name="ld")
        nc.scalar.activation(ld[:], denom[:], AF.Ln)
        # partial = ld - LOGIT_SCALE * pos_raw
        part = epi_pool.tile([P, MB], FP32, tag="part", name="part")
        nc.vector.scalar_tensor_tensor(
            out=part[:], in0=pos[:], scalar=-LOGIT_SCALE, in1=ld[:],
            op0=ALU.mult, op1=ALU.add)
        rowtot = epi_pool.tile([P, 1], FP32, tag="rowtot", name="rowtot")
        nc.vector.tensor_reduce(rowtot[:], part[:], axis=AX.X, op=ALU.add)
        pfin = psum_pool.tile([P, 2 * CB], FP32, tag="ps", name="pfin")
        nc.tensor.matmul(pfin[:1, :1], lhsT=ones[:], rhs=rowtot[:])
        out_sb = epi_pool.tile([1, 1], FP32, tag="osb", name="out_sb")
        nc.vector.tensor_copy(out_sb[:], pfin[:1, :1])
        nc.sync.dma_start(out=out[:, :], in_=out_sb[:])

    with mock.patch("concourse.bacc.get_activation_tables",
                    _filtered_activation_tables):
        nc.compile()
    return nc


_CACHE_LOCK = threading.Lock()
_CACHED_NC = None


def _get_nc():
    global _CACHED_NC
    with _CACHE_LOCK:
        if _CACHED_NC is None:
            _CACHED_NC = _build_kernel()
        return _CACHED_NC


def _run(inputs, trace=False):
    z_i = np.asarray(inputs["z_i"], dtype=np.float32)
    z_j = np.asarray(inputs["z_j"], dtype=np.float32)
    reps = np.concatenate([z_i, z_j], axis=0)
    in_maps = [
        {"reps": np.ascontiguousarray(
            np.roll(reps, -ROWS_PER_CORE * i, axis=0))}
        for i in range(N_CORES)
    ]
    nc = _get_nc()
    res = run_bass_kernel_spmd(nc, in_maps, list(range(N_CORES)), trace=trace)
    partials = [float(res.results[i]["out"][0, 0]) for i in range(N_CORES)]
    loss = np.float32(np.sum(np.asarray(partials, dtype=np.float64)) / TWO_N)
    return loss, res


def kernel(**inputs):
    loss, _ = _run(inputs, trace=False)
    return np.asarray(loss, dtype=np.float32)


# revision 23
# speedup vs baseline: 2.1242x; 2.1242x over previous
"""NT-Xent / SimCLR contrastive loss on 8 Trainium2 NeuronCores.

v6: fp8 DoubleRow matmuls, u16-packed fp8 xbar transpose, (p t) row
layout for single-descriptor DMAs, hoisted loads, DVE/ACT split scaling.

Strategy (data-parallel over rows of the concatenated representations):
  - Host: reps = concat(z_i, z_j) -> [8192, 512] fp32. Core i receives
    reps rolled by -1024*i rows so that *its* 1024 rows sit at rows
    0..1023. SPMD-identical program; positives at col = row + 4096.
  - Row layout "(p t)": group row p*8 + t lives at partition p, tile t,
    so each partition's 8 rows are CONTIGUOUS 16KB in DRAM -> one
    descriptor per partition per group load (8x fewer than "(t p)"),
    and the scratch bounce store is one 2KB run per partition. All
    loads are issued up front (rows pool holds all 8 groups).
  - Per group: n2 via DVE stt square+rowsum; inv = S/sqrt(n2) via ACT
    Ln+Exp (S=16 scales rows into fp8 range); normed fp8 = rows * inv,
    alternating DVE tensor_scalar / ACT Copy per tile; u16-packed
    feature pairs bounce through DRAM scratch (natural row order) and
    xbar-transpose into repsT[kp][g] [128, 1024] u16 (col q = group
    row q; partition c = features 256*kp + {2c, 2c+1} interleaved).
  - Phase B: DoubleRow fp8 matmuls ([128, 2, N] APs, dim1 = the byte
    interleave; lhsT = planar deinterleaved own columns). exp(2*sim) =
    ACT Exp(scale=2/S^2) with fused row-sum. Positives diagonal taken
    from PSUM (natural col order) with an identity-mask stt.
  - Self-similarity: denom = rowsum - e^2 (constant; sim_ii == 1 up to
    fp8 quantization, error ~4e-6 relative on the loss).
  - A dependency-chained dummy LDWEIGHTS per group keeps the PE HAM
    clock gate open through phase A.
  - Host: loss = sum(core partials) / 8192.
"""

import sys
import threading
from unittest import mock

sys.path.insert(0, "/opt/trn_rl_repo")

import numpy as np  # noqa: E402

import concourse.tile as tile  # noqa: E402
from concourse import bacc, mybir  # noqa: E402
from concourse.bass_utils import run_bass_kernel_spmd  # noqa: E402
from concourse.hw_specs import get_activation_tables  # noqa: E402
from concourse.masks import make_identity  # noqa: E402
from contextlib import ExitStack  # noqa: E402

P = 128
D = 512
TWO_N = 8192
N_CORES = 8
ROWS_PER_CORE = TWO_N // N_CORES  # 1024
T_INV = 2.0  # 1 / temperature (0.5)
S = 16.0  # fp8 range scale: normed rows stored as S * x / ||x||
LOGIT_SCALE = T_INV / (S * S)  # psum -> exp argument
E_SELF = float(np.exp(T_INV))  # exp(T_INV * sim_ii), sim_ii == 1

NG = 8  # row/column groups
CB = TWO_N // NG  # 1024 rows per group
TPG = CB // P  # 8 [128, 512] row tiles per group
MB = ROWS_PER_CORE // P  # 8 m-blocks of 128 rows
NKP = 2  # feature-pair chunks (256 features each)
NPB = 4  # psum blocks of 2048 columns (2 groups each)

FP32 = mybir.dt.float32
BF16 = mybir.dt.bfloat16
FP8 = mybir.dt.float8e4
U16 = mybir.dt.uint16
AF = mybir.ActivationFunctionType
ALU = mybir.AluOpType
AX = mybir.AxisListType
DR = mybir.MatmulPerfMode.DoubleRow


def _filtered_activation_tables(arch):
    """Steer every Exp/Ln/Copy activation to the one table set containing
    both Exp and Ln, so the table-load pass cannot thrash between sets."""
    tables = get_activation_tables(arch)
    target = None
    for name, funcs in tables.items():
        if AF.Exp in funcs and AF.Ln in funcs:
            target = name
            break
    if target is None:
        return tables
    steer = {AF.Exp, AF.Ln, AF.Copy, AF.Identity}
    return {
        name: (funcs if name == target else funcs - steer)
        for name, funcs in tables.items()
    }


def _build_kernel():
    nc = bacc.Bacc("TRN2", target_bir_lowering=False, debug=False,
                   num_devices=N_CORES)
    reps = nc.dram_tensor("reps", [TWO_N, D], FP32, kind="ExternalInput").ap()
    out = nc.dram_tensor("out", [1, 1], FP32, kind="ExternalOutput").ap()

    with tile.TileContext(nc) as tc, ExitStack() as ctx:
        rows_pool = ctx.enter_context(tc.tile_pool(name="rows", bufs=1))
        normed_pool = ctx.enter_context(tc.tile_pool(name="normed", bufs=2))
        sq_pool = ctx.enter_context(tc.tile_pool(name="sq", bufs=2))
        stats_pool = ctx.enter_context(tc.tile_pool(name="stats", bufs=1))
        repsT_pool = ctx.enter_context(tc.tile_pool(name="repsT", bufs=1))
        dram_pool = ctx.enter_context(
            tc.tile_pool(name="scratch", bufs=NKP * NG, space="DRAM"))
        psum_pool = ctx.enter_context(
            tc.tile_pool(name="psum", bufs=2, space="PSUM"))
        exp_pool = ctx.enter_context(tc.tile_pool(name="exp", bufs=2))
        junk_pool = ctx.enter_context(tc.tile_pool(name="junk", bufs=2))
        epi_pool = ctx.enter_context(tc.tile_pool(name="epi", bufs=1))

        # --- constants -----------------------------------------------------
        ident = stats_pool.tile([P, P], FP32, tag="ident", name="ident")
        make_identity(nc, ident[:])
        ones = stats_pool.tile([P, 1], FP32, tag="ones", name="ones")
        nc.gpsimd.memset(ones[:], 1.0)

        # accumulators for the main loop
        rs_all = stats_pool.tile([P, MB * NPB], FP32, tag="rs", name="rs_all")
        pos = stats_pool.tile([P, MB], FP32, tag="pos", name="pos")

        # repsT[kp][g]: [128, 1024] u16 — feature-pair chunk kp (features
        # 256*kp + {2c, 2c+1} byte-interleaved at partition c), group col
        # q = group row q (natural order).
        repsT = [[repsT_pool.tile([P, CB], U16, tag=f"rT{kp}_{g}",
                                  name=f"repsT_{kp}_{g}")
                  for g in range(NG)]
                 for kp in range(NKP)]
        # repsT0[kp]: [128, 2048] fp8, planar deinterleave of own columns:
        # layout "p (two q)" so the DoubleRow lhsT slice [128, 2, 128] has
        # a contiguous fast dim.
        repsT0 = [repsT_pool.tile([P, 2 * ROWS_PER_CORE], FP8, tag=f"rTz_{kp}",
                                  name=f"repsT0_{kp}")
                  for kp in range(NKP)]

        # --- all group loads up front (no head-of-line blocking) -----------
        rows_tiles = []
        for g in range(NG):
            rows_g = rows_pool.tile([P, TPG * D], FP32, tag=f"rows{g}",
                                    name=f"rows_{g}")
            rows_tiles.append(rows_g)
            src = reps[g * CB:(g + 1) * CB, :].rearrange(
                "(p t) d -> p t d", t=TPG)
            if g == 0:
                # halves: square pass starts after 8KB/partition arrives
                for h in range(2):
                    hs = TPG // 2
                    nc.sync.dma_start(
                        out=rows_g[:, h * hs * D:(h + 1) * hs * D].rearrange(
                            "p (t d) -> p t d", d=D),
                        in_=src[:, h * hs:(h + 1) * hs, :])
            else:
                nc.sync.dma_start(
                    out=rows_g[:].rearrange("p (t d) -> p t d", d=D), in_=src)

        # --- phase A: normalize rows to fp8, transpose via u16 xbar --------
        for g in range(NG):
            rows_g = rows_tiles[g]
            n2 = stats_pool.tile([P, TPG], FP32, tag="n2", bufs=2,
                                 name=f"n2_{g}")
            for t in range(TPG):
                sq = sq_pool.tile([P, D], BF16, tag="sq", name=f"sq_{g}_{t}")
                rt = rows_g[:, t * D:(t + 1) * D]
                nc.vector.scalar_tensor_tensor(
                    out=sq[:], in0=rt, scalar=1.0, in1=rt,
                    op0=ALU.mult, op1=ALU.mult, accum_out=n2[:, t:t + 1])
            # inv = S * n2**-0.5 = exp(-0.5 * ln(n2 / S^2))
            lnn = stats_pool.tile([P, TPG], FP32, tag="lnn", bufs=2,
                                  name=f"lnn_{g}")
            nc.scalar.activation(lnn[:], n2[:], AF.Ln, scale=1.0 / (S * S))
            inv = stats_pool.tile([P, TPG], FP32, tag="inv", bufs=2,
                                  name=f"inv_{g}")
            nc.scalar.activation(inv[:], lnn[:], AF.Exp, scale=-0.5)

            normed_g = normed_pool.tile([P, TPG * D], FP8, tag="normed",
                                        name=f"normed_{g}")
            for t in range(TPG):
                # row-scaling: mostly DVE (2x single-port fp32), a slice on
                # ACT to keep the per-group feed pace off one engine
                if t % 4 != 3:
                    nc.vector.tensor_scalar(
                        out=normed_g[:, t * D:(t + 1) * D],
                        in0=rows_g[:, t * D:(t + 1) * D],
                        scalar1=inv[:, t:t + 1], scalar2=None, op0=ALU.mult)
                else:
                    nc.scalar.activation(
                        normed_g[:, t * D:(t + 1) * D],
                        rows_g[:, t * D:(t + 1) * D],
                        AF.Copy, scale=inv[:, t:t + 1])
            # HAM keep-warm: dependency-chained LDWEIGHTS (no PSUM write)
            nc.tensor.ldweights(weights=normed_g[:, 0:P], perf_mode=None)

            # u16 view: adjacent feature pairs -> 2-byte units for the xbar.
            # Natural order: scr row p*8 + t = normed (p, t) = group row.
            # ONE contiguous store per group (4KB/partition run on both
            # sides); the two transposes read kp column-slices of it.
            nview = normed_g[:].bitcast(U16).rearrange(
                "p (t e) -> p t e", e=D // 2)
            scr = dram_pool.tile([CB, NKP * P], U16, tag=f"scr_{g}",
                                 name=f"scr_{g}")
            nc.sync.dma_start(
                out=scr[:].rearrange("(p t) c -> p t c", p=P), in_=nview)
            for kp in range(NKP):
                nc.sync.dma_start_transpose(
                    repsT[kp][g][:], scr[:, kp * P:(kp + 1) * P])
            if g == 0:
                # deinterleave own columns into planar lhsT layout:
                # dst[c, i*1024 + q] = src byte 2q + i
                for kp in range(NKP):
                    nc.vector.tensor_copy(
                        repsT0[kp][:].rearrange(
                            "p (two q) -> p two q", two=2),
                        repsT[kp][0][:].bitcast(FP8).rearrange(
                            "p (q two) -> p two q", two=2))

        # --- phase B: DoubleRow similarity matmuls + softmax stats ---------
        for pb in range(NPB):
            for m in range(MB):
                ps = psum_pool.tile([P, 2 * CB], FP32, tag="ps",
                                    name=f"ps_{pb}_{m}")
                for kp in range(NKP):
                    lhsT = repsT0[kp][:].rearrange(
                        "p (two mj) -> p two mj", two=2)[
                        :, :, m * P:(m + 1) * P]
                    for half in range(2):
                        rhs_g = repsT[kp][2 * pb + half][:].bitcast(
                            FP8).rearrange("p (n two) -> p two n", two=2)
                        for ns in range(2):
                            nc.tensor.matmul(
                                ps[:, half * CB + ns * 512:
                                   half * CB + (ns + 1) * 512],
                                lhsT=lhsT,
                                rhs=rhs_g[:, :, ns * 512:(ns + 1) * 512],
                                start=(kp == 0), stop=(kp == NKP - 1),
                                perf_mode=DR, skip_group_check=True)
                et = exp_pool.tile([P, 2 * CB], BF16, tag="et",
                                   name=f"et_{pb}_{m}")
                nc.scalar.activation(
                    et[:], ps[:], AF.Exp, scale=LOGIT_SCALE,
                    accum_out=rs_all[:, m * NPB + pb:m * NPB + pb + 1])
                if pb == 2:
                    # positives: global col = 4096 + row -> group 4 (first
                    # half of this psum block), natural in-group col = row.
                    junk = junk_pool.tile([P, P], FP32, tag="junk",
                                          name=f"junk_p_{m}")
                    nc.vector.scalar_tensor_tensor(
                        out=junk[:], in0=ps[:, m * P:(m + 1) * P],
                        scalar=1.0, in1=ident[:],
                        op0=ALU.mult, op1=ALU.mult,
                        accum_out=pos[:, m:m + 1])

        # --- epilogue ------------------------------------------------------
        sums = epi_pool.tile([P, MB], FP32, tag="sums", name="sums")
        nc.vector.tensor_reduce(
            sums[:], rs_all[:].rearrange("p (m b) -> p m b", b=NPB),
            axis=AX.X, op=ALU.add)
        denom = epi_pool.tile([P, MB], FP32, tag="denom", name="denom")
        nc.vector.tensor_scalar_add(denom[:], sums[:], -E_SELF)
        ld = epi_pool.tile([P, MB], FP32, tag="ld", name="ld")
        nc.scalar.activation(ld[:], denom[:], AF.Ln)
        # partial = ld - LOGIT_SCALE * pos_raw
        part = epi_pool.tile([P, MB], FP32, tag="part", name="part")
        nc.vector.scalar_tensor_tensor(
            out=part[:], in0=pos[:], scalar=-LOGIT_SCALE, in1=ld[:],
            op0=ALU.mult, op1=ALU.add)
        rowtot = epi_pool.tile([P, 1], FP32, tag="rowtot", name="rowtot")
        nc.vector.tensor_reduce(rowtot[:], part[:], axis=AX.X, op=ALU.add)
        pfin = psum_pool.tile([P, 2 * CB], FP32, tag="ps", name="pfin")
        nc.tensor.matmul(pfin[:1, :1], lhsT=ones[:], rhs=rowtot[:])
        out_sb = epi_pool.tile([1, 1], FP32, tag="osb", name="out_sb")
        nc.vector.tensor_copy(out_sb[:], pfin[:1, :1])
        nc.sync.dma_start(out=out[:, :], in_=out_sb[:])

    with mock.patch("concourse.bacc.get_activation_tables",
                    _filtered_activation_tables):
        nc.compile()
    return nc


_CACHE_LOCK = threading.Lock()
_CACHED_NC = None


def _get_nc():
    global _CACHED_NC
    with _CACHE_LOCK:
        if _CACHED_NC is None:
            _CACHED_NC = _build_kernel()
        return _CACHED_NC


def _run(inputs, trace=False):
    z_i = np.asarray(inputs["z_i"], dtype=np.float32)
    z_j = np.asarray(inputs["z_j"], dtype=np.float32)
    reps = np.concatenate([z_i, z_j], axis=0)
    in_maps = [
        {"reps": np.ascontiguousarray(
            np.roll(reps, -ROWS_PER_CORE * i, axis=0))}
        for i in range(N_CORES)
    ]
    nc = _get_nc()
    res = run_bass_kernel_spmd(nc, in_maps, list(range(N_CORES)), trace=trace)
    partials = [float(res.results[i]["out"][0, 0]) for i in range(N_CORES)]
    loss = np.float32(np.sum(np.asarray(partials, dtype=np.float64)) / TWO_N)
    return loss, res


def kernel(**inputs):
    loss, _ = _run(inputs, trace=False)
    return np.asarray(loss, dtype=np.float32)
